# revision 24
# baseline (speedup 1.0000x reference)
"""Trainium2 Bass kernel for nn_Capsule (dynamic routing, 3 iterations).

Reference computation (per batch b, hidden h, routing dim r=64):
  v0 = squash(mean_r x)                      squash(s) = s * ||s||/(1+||s||)
  for u in (v0, v0+v1):
      w   = softmax_r(x * u)                 (softmax over r, per (b,h))
      s   = sum_r w * x
      v   = squash(s)
  return v2                                  shape [B, H]

Sharding: pure data parallel over batch across 8 NeuronCores.

Design notes:
  - x is cast fp32->bf16 *during* the HBM->SBUF DMA (SWDGE cast, free).
  - logits = x*u and prod = e*x run on VectorE as bf16 tensor_tensor; u is
    physically replicated across the chunk's r-slices (urep) so both
    operands are contiguous and the DVE 2x bf16 mode engages (a broadcast
    access pattern drops the op to 1x -- measured).
  - e = exp(logits) on ScalarE (LUT rate is dtype-independent).
  - sum_r reductions are identity-bf16 matmuls accumulating into PSUM
    (fp32), on TensorE.
  - squash / divide: VectorE fp32, bit-hack rsqrt + 2 Newton steps,
    reciprocal_approx_fast for 1/den; small scale ops ride ScalarE.
  - The unit pipeline is staggered: while unit k (one group of 128
    batches) runs its iter-2 chunks, unit k+1's x tiles are DMA-loaded
    and its mean matmuls are interleaved into the TensorE stream, and its
    squash0 is emitted before unit k's finish -- the in-order engine
    queues then have independent work during every serial boundary.
  - Emission is software-pipelined one chunk deep so VectorE does not
    stall on ScalarE's exp latency.
"""

import numpy as np

B, R, H = 2048, 64, 512
N_CORES = 8
BPC = B // N_CORES  # batches per core
P = 128             # partitions (batches per group/unit)

# Tunables
RT = 8              # r-slices per x DMA tile
CH = 8              # r-slices per compute chunk
MEAN_LAG = 2        # chunks between a tile's DMA issue and its mean matmuls

_PROGRAM_CACHE = {}


def _build_program(bpc=BPC, reps=1):
    import concourse.tile as tile
    from concourse import bacc, mybir

    f32 = mybir.dt.float32
    bf16 = mybir.dt.bfloat16
    i32 = mybir.dt.int32
    AF = mybir.ActivationFunctionType
    OP = mybir.AluOpType
    AX = mybir.AxisListType

    G = bpc // P        # groups of 128 batches
    NT = R // RT        # x tiles per group
    CPT = RT // CH if RT >= CH else 1
    NCH = R // CH       # compute chunks per group

    nc = bacc.Bacc(
        "TRN2",
        target_bir_lowering=False,
        debug=False,
        enable_asserts=False,
    )
    x_d = nc.dram_tensor("x", [bpc, R, H], f32, kind="ExternalInput").ap()
    id_d = nc.dram_tensor("ident", [P, P], f32, kind="ExternalInput").ap()
    out_d = nc.dram_tensor("out", [bpc, H], f32, kind="ExternalOutput").ap()

    with tile.TileContext(nc) as tc:
        with (
            tc.tile_pool(name="xbp", bufs=NT + 2) as xbp,
            tc.tile_pool(name="lgp", bufs=3) as lgp,
            tc.tile_pool(name="epp", bufs=4) as epp,
            tc.tile_pool(name="ppp", bufs=3) as ppp,
            tc.tile_pool(name="urp", bufs=2) as urp,
            tc.tile_pool(name="dnp", bufs=2) as dnp,
            tc.tile_pool(name="cst", bufs=1) as cst,
            tc.tile_pool(name="outp", bufs=2) as outp,
            tc.tile_pool(name="psp", bufs=2, space="PSUM") as psp,
        ):
            identb = cst.tile([P, P], bf16)
            nc.gpsimd.dma_start(identb[:], id_d)  # cast f32 -> bf16
            magic = cst.tile([P, 1], i32)
            nc.vector.memset(magic[:], 0x5F3759DF)

            def squash_factor(s_ap, tag):
                """gsc[p,1] = sn/(1+sn) = 1/(1+rsqrt(nrm)), sn=||s||_2.

                rsqrt via bit-hack seed + 2 Newton iterations, VectorE only
                (avoids ScalarE act-table switches)."""
                sq = dnp.tile([P, H], f32, name=f"sq_{tag}", tag="sq")
                nrm = dnp.tile([P, 1], f32, name=f"nrm_{tag}", tag="nrm")
                # Square on ScalarE with fused free-dim accumulate: nrm comes
                # out of the same instruction, dropping a serial VectorE
                # reduce from the squash critical path
                nc.scalar.activation(sq[:], s_ap, AF.Square, accum_out=nrm[:])
                half_i = dnp.tile([P, 1], i32, name=f"hi_{tag}", tag="hi")
                nc.vector.tensor_scalar(
                    half_i[:], nrm[:].bitcast(i32), 1, None,
                    op0=OP.arith_shift_right,
                )
                y0 = dnp.tile([P, 1], i32, name=f"y0_{tag}", tag="y0")
                nc.vector.scalar_tensor_tensor(
                    y0[:], magic[:], 0, half_i[:],
                    op0=OP.bypass, op1=OP.subtract,
                )
                y = y0[:].bitcast(f32)
                for nr in range(2):
                    # t2 = y*y*nrm in one op: (y mult nrm[P,1]) mult y
                    t2 = dnp.tile([P, 1], f32, name=f"t2_{tag}_{nr}", tag="t2")
                    nc.vector.scalar_tensor_tensor(
                        t2[:], y, nrm[:], y, op0=OP.mult, op1=OP.mult
                    )
                    t3 = dnp.tile([P, 1], f32, name=f"t3_{tag}_{nr}", tag="t3")
                    nc.vector.tensor_scalar(
                        t3[:], t2[:], -0.5, 1.5, op0=OP.mult, op1=OP.add
                    )
                    yn = dnp.tile([P, 1], f32, name=f"y_{tag}_{nr}", tag="yn")
                    nc.vector.tensor_mul(yn[:], y, t3[:])
                    y = yn[:]
                y1 = dnp.tile([P, 1], f32, name=f"y1_{tag}", tag="y1")
                nc.vector.tensor_scalar_add(y1[:], y, 1.0)
                gsc = dnp.tile([P, 1], f32, name=f"gsc_{tag}", tag="gsc")
                nc.vector.reciprocal(gsc[:], y1[:])
                return gsc

            units = [(rep, g) for rep in range(reps) for g in range(G)]
            NU = len(units)
            state = {}

            def emit_load(k, t):
                rep, g = units[k]
                st = state.setdefault(k, {"xt": [None] * NT})
                x_t = xbp.tile([P, RT, H], bf16, name="xtile", tag="xtile")
                nc.gpsimd.dma_start(
                    x_t[:], x_d[g * P:(g + 1) * P, t * RT:(t + 1) * RT, :]
                )
                st["xt"][t] = x_t

            def emit_mean_tile(k, t):
                st = state[k]
                if t == 0:
                    st["mean"] = psp.tile(
                        [P, H], f32, name=f"mean_{k}", tag="mean")
                for r in range(RT):
                    nc.tensor.matmul(
                        st["mean"][:],
                        identb[:],
                        st["xt"][t][:, r, :],
                        start=(t == 0 and r == 0),
                        stop=(t == NT - 1 and r == RT - 1),
                    )

            def emit_squash0(k):
                st = state[k]
                s0 = dnp.tile([P, H], f32, name=f"s0_{k}", tag="s0")
                nc.scalar.mul(s0[:], st["mean"][:], 1.0 / R)
                gsc0 = squash_factor(s0[:], f"sq0_{k}")
                v0 = dnp.tile([P, H], f32, name=f"v0_{k}", tag="v0")
                nc.scalar.mul(v0[:], s0[:], gsc0[:])
                u = dnp.tile([P, H], bf16, name=f"u_{k}", tag="u")
                nc.scalar.mul(u[:], s0[:], gsc0[:])
                st["v0"], st["u"] = v0, u

            def xs_of(k, ci):
                t, c = divmod(ci, CPT)
                return state[k]["xt"][t][:, c * CH:(c + 1) * CH, :]

            # prologue: unit 0 fully staged
            for t in range(NT):
                emit_load(0, t)
            for t in range(NT):
                emit_mean_tile(0, t)
            emit_squash0(0)

            for k in range(NU):
                st = state[k]
                for it in (1, 2):
                    stage_next = (it == 2 and k + 1 < NU)
                    den_ps = psp.tile(
                        [P, H], f32, name=f"den_{k}_{it}", tag="den")
                    num_ps = psp.tile(
                        [P, H], f32, name=f"num_{k}_{it}", tag="num")
                    # replicate u across the chunk's r-slices: contiguous
                    # (non-broadcast) operands keep the DVE 2x bf16 mode
                    urep = urp.tile(
                        [P, CH, H], bf16, name=f"ur_{k}_{it}", tag="ur")
                    nc.vector.tensor_copy(urep[:, 0, :], st["u"][:])
                    d = 1
                    while d < CH:
                        nc.vector.tensor_copy(
                            urep[:, d:min(2 * d, CH), :], urep[:, 0:d, :]
                        )
                        d *= 2
                    eps = [None] * NCH
                    for ci in range(NCH + 1):
                        if stage_next and ci < NT:
                            emit_load(k + 1, ci)
                        if ci < NCH:
                            lg = lgp.tile(
                                [P, CH, H], bf16, name="lg", tag="lg")
                            nc.vector.tensor_tensor(
                                lg[:], xs_of(k, ci), urep[:], op=OP.mult
                            )
                            ep = epp.tile(
                                [P, CH, H], bf16, name="ep", tag="ep")
                            nc.scalar.activation(ep[:], lg[:], AF.Exp)
                            eps[ci] = ep
                        if ci > 0:
                            pi = ci - 1
                            epv = eps[pi]
                            eps[pi] = None
                            pp = ppp.tile(
                                [P, CH, H], bf16, name="pp", tag="pp")
                            nc.vector.tensor_tensor(
                                pp[:], epv[:], xs_of(k, pi), op=OP.mult
                            )
                            for r in range(CH):
                                nc.tensor.matmul(
                                    den_ps[:],
                                    identb[:],
                                    epv[:, r, :],
                                    start=(pi == 0 and r == 0),
                                    stop=(pi == NCH - 1 and r == CH - 1),
                                )
                            for r in range(CH):
                                nc.tensor.matmul(
                                    num_ps[:],
                                    identb[:],
                                    pp[:, r, :],
                                    start=(pi == 0 and r == 0),
                                    stop=(pi == NCH - 1 and r == CH - 1),
                                )
                        if stage_next and MEAN_LAG <= ci < NT + MEAN_LAG:
                            t_m = ci - MEAN_LAG
                            if t_m < NT:
                                emit_mean_tile(k + 1, t_m)
                    if stage_next:
                        # flush mean tiles the chunk loop didn't reach
                        for t_m in range(max(0, NCH + 1 - MEAN_LAG), NT):
                            emit_mean_tile(k + 1, t_m)
                        emit_squash0(k + 1)
                    rd = dnp.tile([P, H], f32, name=f"rd_{k}_{it}", tag="rd")
                    nc.vector.reciprocal_approx_fast(rd[:], den_ps[:])
                    s = dnp.tile([P, H], f32, name=f"s_{k}_{it}", tag="s")
                    nc.vector.tensor_mul(s[:], num_ps[:], rd[:])
                    gsc = squash_factor(s[:], f"f_{k}_{it}")
                    if it == 1:
                        u2 = dnp.tile([P, H], bf16, name=f"u2_{k}", tag="u2")
                        # u2 = s*gsc + v0
                        nc.vector.scalar_tensor_tensor(
                            u2[:], s[:], gsc[:], st["v0"][:],
                            op0=OP.mult, op1=OP.add,
                        )
                        st["u"] = u2
                    else:
                        rep, g = units[k]
                        o = outp.tile([P, H], f32, name="o", tag="o")
                        nc.scalar.mul(o[:], s[:], gsc[:])
                        nc.sync.dma_start(out_d[g * P:(g + 1) * P, :], o[:])
                        state[k] = None  # release python refs

    nc.compile()
    return nc


def _get_program(bpc=BPC, reps=1):
    key = (bpc, reps)
    if key not in _PROGRAM_CACHE:
        _PROGRAM_CACHE[key] = _build_program(bpc, reps)
    return _PROGRAM_CACHE[key]


def _identity_np():
    return np.eye(P, dtype=np.float32)


def kernel(input_matrix: np.ndarray) -> np.ndarray:
    from concourse.bass_utils import run_bass_kernel_spmd

    x = np.ascontiguousarray(np.asarray(input_matrix, dtype=np.float32))
    assert x.shape == (B, R, H)
    nc = _get_program()
    ident = _identity_np()
    shards = x.reshape(N_CORES, BPC, R, H)
    in_maps = [
        {"x": np.ascontiguousarray(shards[i]), "ident": ident}
        for i in range(N_CORES)
    ]
    res = run_bass_kernel_spmd(nc, in_maps, core_ids=list(range(N_CORES)))
    out = np.concatenate(
        [res.results[i]["out"] for i in range(N_CORES)], axis=0
    )
    return out


if __name__ == "__main__":
    nc = _get_program()
    print("program built and compiled OK")


# revision 25
# speedup vs baseline: 1.0420x; 1.0420x over previous
"""Trainium2 Bass kernel for nn_Capsule (dynamic routing, 3 iterations).

Reference computation (per batch b, hidden h, routing dim r=64):
  v0 = squash(mean_r x)                      squash(s) = s * ||s||/(1+||s||)
  for u in (v0, v0+v1):
      w   = softmax_r(x * u)                 (softmax over r, per (b,h))
      s   = sum_r w * x
      v   = squash(s)
  return v2                                  shape [B, H]

Sharding: pure data parallel over batch across 8 NeuronCores.

Design notes:
  - x is cast fp32->bf16 *during* the HBM->SBUF DMA (SWDGE cast, free).
  - logits = x*u and prod = e*x run on VectorE as bf16 tensor_tensor; u is
    physically replicated across the chunk's r-slices (urep) so both
    operands are contiguous and the DVE 2x bf16 mode engages (a broadcast
    access pattern drops the op to 1x -- measured).
  - e = exp(logits) on ScalarE (LUT rate is dtype-independent).
  - sum_r reductions are identity-bf16 matmuls accumulating into PSUM
    (fp32), on TensorE.
  - squash / divide: VectorE fp32, bit-hack rsqrt + 2 Newton steps,
    reciprocal_approx_fast for 1/den; small scale ops ride ScalarE.
  - The unit pipeline is staggered: while unit k (one group of 128
    batches) runs its iter-2 chunks, unit k+1's x tiles are DMA-loaded
    and its mean matmuls are interleaved into the TensorE stream, and its
    squash0 is emitted before unit k's finish -- the in-order engine
    queues then have independent work during every serial boundary.
  - Emission is software-pipelined one chunk deep so VectorE does not
    stall on ScalarE's exp latency.
"""

import numpy as np

B, R, H = 2048, 64, 512
N_CORES = 8
BPC = B // N_CORES  # batches per core
P = 128             # partitions (batches per group/unit)

# Tunables
RT = 8              # r-slices per x DMA tile
CH = 8              # r-slices per compute chunk
MEAN_LAG = 2        # chunks between a tile's DMA issue and its mean matmuls

_PROGRAM_CACHE = {}


def _build_program(bpc=BPC, reps=1):
    import concourse.tile as tile
    from concourse import bacc, mybir

    f32 = mybir.dt.float32
    bf16 = mybir.dt.bfloat16
    i32 = mybir.dt.int32
    AF = mybir.ActivationFunctionType
    OP = mybir.AluOpType
    AX = mybir.AxisListType

    G = bpc // P        # groups of 128 batches
    NT = R // RT        # x tiles per group
    CPT = RT // CH if RT >= CH else 1
    NCH = R // CH       # compute chunks per group

    nc = bacc.Bacc(
        "TRN2",
        target_bir_lowering=False,
        debug=False,
        enable_asserts=False,
    )
    x_d = nc.dram_tensor("x", [bpc, R, H], f32, kind="ExternalInput").ap()
    id_d = nc.dram_tensor("ident", [P, P], f32, kind="ExternalInput").ap()
    out_d = nc.dram_tensor("out", [bpc, H], f32, kind="ExternalOutput").ap()

    with tile.TileContext(nc) as tc:
        with (
            tc.tile_pool(name="xbp", bufs=NT + 2) as xbp,
            tc.tile_pool(name="lgp", bufs=3) as lgp,
            tc.tile_pool(name="epp", bufs=4) as epp,
            tc.tile_pool(name="ppp", bufs=3) as ppp,
            tc.tile_pool(name="urp", bufs=2) as urp,
            tc.tile_pool(name="dnp", bufs=2) as dnp,
            tc.tile_pool(name="cst", bufs=1) as cst,
            tc.tile_pool(name="outp", bufs=2) as outp,
            tc.tile_pool(name="psp", bufs=2, space="PSUM") as psp,
        ):
            identb = cst.tile([P, P], bf16)
            nc.gpsimd.dma_start(identb[:], id_d)  # cast f32 -> bf16
            magic = cst.tile([P, 1], i32)
            nc.vector.memset(magic[:], 0x5F3759DF)

            def squash_factor(s_ap, tag):
                """gsc[p,1] = sn/(1+sn) = 1/(1+rsqrt(nrm)), sn=||s||_2.

                rsqrt via bit-hack seed + 2 Newton iterations, VectorE only
                (avoids ScalarE act-table switches)."""
                sq = dnp.tile([P, H], f32, name=f"sq_{tag}", tag="sq")
                nrm = dnp.tile([P, 1], f32, name=f"nrm_{tag}", tag="nrm")
                # Square on ScalarE with fused free-dim accumulate: nrm comes
                # out of the same instruction, dropping a serial VectorE
                # reduce from the squash critical path
                nc.scalar.activation(sq[:], s_ap, AF.Square, accum_out=nrm[:])
                half_i = dnp.tile([P, 1], i32, name=f"hi_{tag}", tag="hi")
                nc.vector.tensor_scalar(
                    half_i[:], nrm[:].bitcast(i32), 1, None,
                    op0=OP.arith_shift_right,
                )
                y0 = dnp.tile([P, 1], i32, name=f"y0_{tag}", tag="y0")
                nc.vector.scalar_tensor_tensor(
                    y0[:], magic[:], 0, half_i[:],
                    op0=OP.bypass, op1=OP.subtract,
                )
                y = y0[:].bitcast(f32)
                for nr in range(2):
                    # t2 = y*y*nrm in one op: (y mult nrm[P,1]) mult y
                    t2 = dnp.tile([P, 1], f32, name=f"t2_{tag}_{nr}", tag="t2")
                    nc.vector.scalar_tensor_tensor(
                        t2[:], y, nrm[:], y, op0=OP.mult, op1=OP.mult
                    )
                    t3 = dnp.tile([P, 1], f32, name=f"t3_{tag}_{nr}", tag="t3")
                    nc.vector.tensor_scalar(
                        t3[:], t2[:], -0.5, 1.5, op0=OP.mult, op1=OP.add
                    )
                    yn = dnp.tile([P, 1], f32, name=f"y_{tag}_{nr}", tag="yn")
                    nc.vector.tensor_mul(yn[:], y, t3[:])
                    y = yn[:]
                y1 = dnp.tile([P, 1], f32, name=f"y1_{tag}", tag="y1")
                nc.vector.tensor_scalar_add(y1[:], y, 1.0)
                gsc = dnp.tile([P, 1], f32, name=f"gsc_{tag}", tag="gsc")
                nc.vector.reciprocal(gsc[:], y1[:])
                return gsc

            units = [(rep, g) for rep in range(reps) for g in range(G)]
            NU = len(units)
            state = {}

            def emit_load(k, t):
                rep, g = units[k]
                st = state.setdefault(k, {"xt": [None] * NT})
                x_t = xbp.tile([P, RT, H], bf16, name="xtile", tag="xtile")
                nc.gpsimd.dma_start(
                    x_t[:], x_d[g * P:(g + 1) * P, t * RT:(t + 1) * RT, :]
                )
                st["xt"][t] = x_t

            def emit_mean_tile(k, t):
                st = state[k]
                if t == 0:
                    st["mean"] = psp.tile(
                        [P, H], f32, name=f"mean_{k}", tag="mean")
                for r in range(RT):
                    nc.tensor.matmul(
                        st["mean"][:],
                        identb[:],
                        st["xt"][t][:, r, :],
                        start=(t == 0 and r == 0),
                        stop=(t == NT - 1 and r == RT - 1),
                    )

            def build_urep(k, it, u_ap):
                # replicate u across the chunk's r-slices: contiguous
                # (non-broadcast) operands keep the DVE 2x bf16 mode; built
                # as soon as u exists so it is off the iteration boundary
                ur = urp.tile([P, CH, H], bf16, name=f"ur_{k}_{it}", tag="ur")
                nc.vector.tensor_copy(ur[:, 0, :], u_ap)
                d = 1
                while d < CH:
                    nc.vector.tensor_copy(
                        ur[:, d:min(2 * d, CH), :], ur[:, 0:d, :]
                    )
                    d *= 2
                state[k][f"urep{it}"] = ur

            def emit_squash0(k):
                st = state[k]
                s0 = dnp.tile([P, H], f32, name=f"s0_{k}", tag="s0")
                nc.scalar.mul(s0[:], st["mean"][:], 1.0 / R)
                gsc0 = squash_factor(s0[:], f"sq0_{k}")
                v0 = dnp.tile([P, H], f32, name=f"v0_{k}", tag="v0")
                nc.scalar.mul(v0[:], s0[:], gsc0[:])
                u = dnp.tile([P, H], bf16, name=f"u_{k}", tag="u")
                nc.scalar.mul(u[:], s0[:], gsc0[:])
                st["v0"], st["u"] = v0, u
                build_urep(k, 1, u[:])

            def xs_of(k, ci):
                t, c = divmod(ci, CPT)
                return state[k]["xt"][t][:, c * CH:(c + 1) * CH, :]

            # prologue: unit 0 fully staged
            for t in range(NT):
                emit_load(0, t)
            for t in range(NT):
                emit_mean_tile(0, t)
            emit_squash0(0)

            for k in range(NU):
                st = state[k]
                for it in (1, 2):
                    stage_next = (it == 2 and k + 1 < NU)
                    den_ps = psp.tile(
                        [P, H], f32, name=f"den_{k}_{it}", tag="den")
                    num_ps = psp.tile(
                        [P, H], f32, name=f"num_{k}_{it}", tag="num")
                    urep = st[f"urep{it}"]
                    eps = [None] * NCH
                    for ci in range(NCH + 1):
                        if stage_next and ci < NT:
                            emit_load(k + 1, ci)
                        if ci < NCH:
                            lg = lgp.tile(
                                [P, CH, H], bf16, name="lg", tag="lg")
                            nc.vector.tensor_tensor(
                                lg[:], xs_of(k, ci), urep[:], op=OP.mult
                            )
                            ep = epp.tile(
                                [P, CH, H], bf16, name="ep", tag="ep")
                            nc.scalar.activation(ep[:], lg[:], AF.Exp)
                            eps[ci] = ep
                        if ci > 0:
                            pi = ci - 1
                            epv = eps[pi]
                            eps[pi] = None
                            pp = ppp.tile(
                                [P, CH, H], bf16, name="pp", tag="pp")
                            nc.vector.tensor_tensor(
                                pp[:], epv[:], xs_of(k, pi), op=OP.mult
                            )
                            for r in range(CH):
                                nc.tensor.matmul(
                                    den_ps[:],
                                    identb[:],
                                    epv[:, r, :],
                                    start=(pi == 0 and r == 0),
                                    stop=(pi == NCH - 1 and r == CH - 1),
                                )
                            for r in range(CH):
                                nc.tensor.matmul(
                                    num_ps[:],
                                    identb[:],
                                    pp[:, r, :],
                                    start=(pi == 0 and r == 0),
                                    stop=(pi == NCH - 1 and r == CH - 1),
                                )
                        if stage_next and MEAN_LAG <= ci < NT + MEAN_LAG:
                            t_m = ci - MEAN_LAG
                            if t_m < NT:
                                emit_mean_tile(k + 1, t_m)
                    if stage_next:
                        # flush mean tiles the chunk loop didn't reach
                        for t_m in range(max(0, NCH + 1 - MEAN_LAG), NT):
                            emit_mean_tile(k + 1, t_m)
                        emit_squash0(k + 1)
                    rd = dnp.tile([P, H], f32, name=f"rd_{k}_{it}", tag="rd")
                    nc.vector.reciprocal_approx_fast(rd[:], den_ps[:])
                    s = dnp.tile([P, H], f32, name=f"s_{k}_{it}", tag="s")
                    nc.vector.tensor_mul(s[:], num_ps[:], rd[:])
                    gsc = squash_factor(s[:], f"f_{k}_{it}")
                    if it == 1:
                        u2 = dnp.tile([P, H], bf16, name=f"u2_{k}", tag="u2")
                        # u2 = s*gsc + v0
                        nc.vector.scalar_tensor_tensor(
                            u2[:], s[:], gsc[:], st["v0"][:],
                            op0=OP.mult, op1=OP.add,
                        )
                        st["u"] = u2
                        build_urep(k, 2, u2[:])
                    else:
                        rep, g = units[k]
                        o = outp.tile([P, H], f32, name="o", tag="o")
                        nc.scalar.mul(o[:], s[:], gsc[:])
                        nc.sync.dma_start(out_d[g * P:(g + 1) * P, :], o[:])
                        state[k] = None  # release python refs

    nc.compile()
    return nc


def _get_program(bpc=BPC, reps=1):
    key = (bpc, reps)
    if key not in _PROGRAM_CACHE:
        _PROGRAM_CACHE[key] = _build_program(bpc, reps)
    return _PROGRAM_CACHE[key]


def _identity_np():
    return np.eye(P, dtype=np.float32)


def kernel(input_matrix: np.ndarray) -> np.ndarray:
    from concourse.bass_utils import run_bass_kernel_spmd

    x = np.ascontiguousarray(np.asarray(input_matrix, dtype=np.float32))
    assert x.shape == (B, R, H)
    nc = _get_program()
    ident = _identity_np()
    shards = x.reshape(N_CORES, BPC, R, H)
    in_maps = [
        {"x": np.ascontiguousarray(shards[i]), "ident": ident}
        for i in range(N_CORES)
    ]
    res = run_bass_kernel_spmd(nc, in_maps, core_ids=list(range(N_CORES)))
    out = np.concatenate(
        [res.results[i]["out"] for i in range(N_CORES)], axis=0
    )
    return out


if __name__ == "__main__":
    nc = _get_program()
    print("program built and compiled OK")


# revision 26
# speedup vs baseline: 1.0943x; 1.0502x over previous
"""Trainium2 Bass kernel for nn_Capsule (dynamic routing, 3 iterations).

Reference computation (per batch b, hidden h, routing dim r=64):
  v0 = squash(mean_r x)                      squash(s) = s * ||s||/(1+||s||)
  for u in (v0, v0+v1):
      w   = softmax_r(x * u)                 (softmax over r, per (b,h))
      s   = sum_r w * x
      v   = squash(s)
  return v2                                  shape [B, H]

Sharding: pure data parallel over batch across 8 NeuronCores.

Design notes:
  - x is cast fp32->bf16 *during* the HBM->SBUF DMA (SWDGE cast, free).
  - logits = x*u and prod = e*x run on VectorE as bf16 tensor_tensor; u is
    physically replicated across the chunk's r-slices (urep) so both
    operands are contiguous and the DVE 2x bf16 mode engages (a broadcast
    access pattern drops the op to 1x -- measured).
  - e = exp(logits) on ScalarE (LUT rate is dtype-independent).
  - sum_r reductions are identity-bf16 matmuls accumulating into PSUM
    (fp32), on TensorE.
  - squash / divide: VectorE fp32, bit-hack rsqrt + 2 Newton steps,
    reciprocal_approx_fast for 1/den; small scale ops ride ScalarE.
  - The unit pipeline is staggered: while unit k (one group of 128
    batches) runs its iter-2 chunks, unit k+1's x tiles are DMA-loaded
    and its mean matmuls are interleaved into the TensorE stream, and its
    squash0 is emitted before unit k's finish -- the in-order engine
    queues then have independent work during every serial boundary.
  - Emission is software-pipelined one chunk deep so VectorE does not
    stall on ScalarE's exp latency.
"""

import numpy as np

B, R, H = 2048, 64, 512
N_CORES = 8
BPC = B // N_CORES  # batches per core
P = 128             # partitions (batches per group/unit)

# Tunables
RT = 8              # r-slices per x DMA tile
CH = 8              # r-slices per compute chunk
MEAN_LAG = 2        # chunks between a tile's DMA issue and its mean matmuls

_PROGRAM_CACHE = {}


def _build_program(bpc=BPC, reps=1):
    import concourse.tile as tile
    from concourse import bacc, mybir

    f32 = mybir.dt.float32
    bf16 = mybir.dt.bfloat16
    i32 = mybir.dt.int32
    AF = mybir.ActivationFunctionType
    OP = mybir.AluOpType
    AX = mybir.AxisListType

    G = bpc // P        # groups of 128 batches
    NT = R // RT        # x tiles per group
    CPT = RT // CH if RT >= CH else 1
    NCH = R // CH       # compute chunks per group

    nc = bacc.Bacc(
        "TRN2",
        target_bir_lowering=False,
        debug=False,
        enable_asserts=False,
    )
    x_d = nc.dram_tensor("x", [bpc, R, H], f32, kind="ExternalInput").ap()
    id_d = nc.dram_tensor("ident", [P, P], f32, kind="ExternalInput").ap()
    out_d = nc.dram_tensor("out", [bpc, H], f32, kind="ExternalOutput").ap()

    with tile.TileContext(nc) as tc:
        with (
            tc.tile_pool(name="xbp", bufs=NT + 2) as xbp,
            tc.tile_pool(name="lgp", bufs=3) as lgp,
            tc.tile_pool(name="epp", bufs=4) as epp,
            tc.tile_pool(name="ppp", bufs=3) as ppp,
            tc.tile_pool(name="urp", bufs=2) as urp,
            tc.tile_pool(name="dnp", bufs=2) as dnp,
            tc.tile_pool(name="cst", bufs=1) as cst,
            tc.tile_pool(name="outp", bufs=2) as outp,
            tc.tile_pool(name="psp", bufs=2, space="PSUM") as psp,
        ):
            identb = cst.tile([P, P], bf16)
            nc.gpsimd.dma_start(identb[:], id_d)  # cast f32 -> bf16
            magic = cst.tile([P, 1], i32)
            nc.vector.memset(magic[:], 0x5F3759DF)

            def squash_factor(s_ap, tag):
                """gsc[p,1] = sn/(1+sn) = 1/(1+rsqrt(nrm)), sn=||s||_2.

                rsqrt via bit-hack seed + 2 Newton iterations, VectorE only
                (avoids ScalarE act-table switches)."""
                sq = dnp.tile([P, H], f32, name=f"sq_{tag}", tag="sq")
                nrm = dnp.tile([P, 1], f32, name=f"nrm_{tag}", tag="nrm")
                # Square on ScalarE with fused free-dim accumulate: nrm comes
                # out of the same instruction, dropping a serial VectorE
                # reduce from the squash critical path
                nc.scalar.activation(sq[:], s_ap, AF.Square, accum_out=nrm[:])
                half_i = dnp.tile([P, 1], i32, name=f"hi_{tag}", tag="hi")
                nc.vector.tensor_scalar(
                    half_i[:], nrm[:].bitcast(i32), 1, None,
                    op0=OP.arith_shift_right,
                )
                y0 = dnp.tile([P, 1], i32, name=f"y0_{tag}", tag="y0")
                nc.vector.scalar_tensor_tensor(
                    y0[:], magic[:], 0, half_i[:],
                    op0=OP.bypass, op1=OP.subtract,
                )
                y = y0[:].bitcast(f32)
                for nr in range(2):
                    # t2 = y*y*nrm in one op: (y mult nrm[P,1]) mult y
                    t2 = dnp.tile([P, 1], f32, name=f"t2_{tag}_{nr}", tag="t2")
                    nc.vector.scalar_tensor_tensor(
                        t2[:], y, nrm[:], y, op0=OP.mult, op1=OP.mult
                    )
                    t3 = dnp.tile([P, 1], f32, name=f"t3_{tag}_{nr}", tag="t3")
                    nc.vector.tensor_scalar(
                        t3[:], t2[:], -0.5, 1.5, op0=OP.mult, op1=OP.add
                    )
                    yn = dnp.tile([P, 1], f32, name=f"y_{tag}_{nr}", tag="yn")
                    nc.vector.tensor_mul(yn[:], y, t3[:])
                    y = yn[:]
                y1 = dnp.tile([P, 1], f32, name=f"y1_{tag}", tag="y1")
                nc.vector.tensor_scalar_add(y1[:], y, 1.0)
                gsc = dnp.tile([P, 1], f32, name=f"gsc_{tag}", tag="gsc")
                nc.vector.reciprocal(gsc[:], y1[:])
                return gsc

            units = [(rep, g) for rep in range(reps) for g in range(G)]
            NU = len(units)
            state = {}

            def emit_load(k, t):
                rep, g = units[k]
                st = state.setdefault(k, {"xt": [None] * NT})
                x_t = xbp.tile([P, RT, H], bf16, name="xtile", tag="xtile")
                nc.gpsimd.dma_start(
                    x_t[:], x_d[g * P:(g + 1) * P, t * RT:(t + 1) * RT, :]
                )
                st["xt"][t] = x_t

            def emit_mean_tile(k, t):
                st = state[k]
                if t == 0:
                    st["mean"] = psp.tile(
                        [P, H], f32, name=f"mean_{k}", tag="mean")
                for r in range(RT):
                    nc.tensor.matmul(
                        st["mean"][:],
                        identb[:],
                        st["xt"][t][:, r, :],
                        start=(t == 0 and r == 0),
                        stop=(t == NT - 1 and r == RT - 1),
                    )

            def build_urep(k, it, u_ap):
                # replicate u across the chunk's r-slices: contiguous
                # (non-broadcast) operands keep the DVE 2x bf16 mode; built
                # as soon as u exists so it is off the iteration boundary
                ur = urp.tile([P, CH, H], bf16, name=f"ur_{k}_{it}", tag="ur")
                nc.vector.tensor_copy(ur[:, 0, :], u_ap)
                d = 1
                while d < CH:
                    nc.vector.tensor_copy(
                        ur[:, d:min(2 * d, CH), :], ur[:, 0:d, :]
                    )
                    d *= 2
                state[k][f"urep{it}"] = ur

            def emit_squash0(k):
                st = state[k]
                s0 = dnp.tile([P, H], f32, name=f"s0_{k}", tag="s0")
                nc.scalar.mul(s0[:], st["mean"][:], 1.0 / R)
                gsc0 = squash_factor(s0[:], f"sq0_{k}")
                v0 = dnp.tile([P, H], f32, name=f"v0_{k}", tag="v0")
                nc.scalar.mul(v0[:], s0[:], gsc0[:])
                u = dnp.tile([P, H], bf16, name=f"u_{k}", tag="u")
                nc.scalar.mul(u[:], s0[:], gsc0[:])
                st["v0"], st["u"] = v0, u
                build_urep(k, 1, u[:])

            def xs_of(k, ci):
                t, c = divmod(ci, CPT)
                return state[k]["xt"][t][:, c * CH:(c + 1) * CH, :]

            # prologue: unit 0 fully staged
            for t in range(NT):
                emit_load(0, t)
            for t in range(NT):
                emit_mean_tile(0, t)
            emit_squash0(0)

            for k in range(NU):
                st = state[k]
                for it in (1, 2):
                    stage_next = (it == 2 and k + 1 < NU)
                    den_ps = psp.tile(
                        [P, H], f32, name=f"den_{k}_{it}", tag="den")
                    num_ps = psp.tile(
                        [P, H], f32, name=f"num_{k}_{it}", tag="num")
                    urep = st[f"urep{it}"]
                    eps = [None] * NCH
                    for ci in range(NCH + 1):
                        if stage_next and ci < NT:
                            emit_load(k + 1, ci)
                        if ci < NCH:
                            lg = lgp.tile(
                                [P, CH, H], bf16, name="lg", tag="lg")
                            nc.vector.tensor_tensor(
                                lg[:], xs_of(k, ci), urep[:], op=OP.mult
                            )
                            ep = epp.tile(
                                [P, CH, H], bf16, name="ep", tag="ep")
                            nc.scalar.activation(ep[:], lg[:], AF.Exp)
                            eps[ci] = ep
                            # den matmuls emitted a chunk ahead of num: they
                            # depend only on ep (ScalarE), so TensorE stays
                            # fed while VectorE finishes the prod for chunk-1
                            for r in range(CH):
                                nc.tensor.matmul(
                                    den_ps[:],
                                    identb[:],
                                    ep[:, r, :],
                                    start=(ci == 0 and r == 0),
                                    stop=(ci == NCH - 1 and r == CH - 1),
                                )
                        if ci > 0:
                            pi = ci - 1
                            epv = eps[pi]
                            eps[pi] = None
                            pp = ppp.tile(
                                [P, CH, H], bf16, name="pp", tag="pp")
                            nc.vector.tensor_tensor(
                                pp[:], epv[:], xs_of(k, pi), op=OP.mult
                            )
                            for r in range(CH):
                                nc.tensor.matmul(
                                    num_ps[:],
                                    identb[:],
                                    pp[:, r, :],
                                    start=(pi == 0 and r == 0),
                                    stop=(pi == NCH - 1 and r == CH - 1),
                                )
                        if stage_next and MEAN_LAG <= ci < NT + MEAN_LAG:
                            t_m = ci - MEAN_LAG
                            if t_m < NT:
                                emit_mean_tile(k + 1, t_m)
                    if stage_next:
                        # flush mean tiles the chunk loop didn't reach
                        for t_m in range(max(0, NCH + 1 - MEAN_LAG), NT):
                            emit_mean_tile(k + 1, t_m)
                        emit_squash0(k + 1)
                    rd = dnp.tile([P, H], f32, name=f"rd_{k}_{it}", tag="rd")
                    nc.vector.reciprocal_approx_fast(rd[:], den_ps[:])
                    s = dnp.tile([P, H], f32, name=f"s_{k}_{it}", tag="s")
                    nc.vector.tensor_mul(s[:], num_ps[:], rd[:])
                    gsc = squash_factor(s[:], f"f_{k}_{it}")
                    if it == 1:
                        u2 = dnp.tile([P, H], bf16, name=f"u2_{k}", tag="u2")
                        # u2 = s*gsc + v0
                        nc.vector.scalar_tensor_tensor(
                            u2[:], s[:], gsc[:], st["v0"][:],
                            op0=OP.mult, op1=OP.add,
                        )
                        st["u"] = u2
                        build_urep(k, 2, u2[:])
                    else:
                        rep, g = units[k]
                        o = outp.tile([P, H], f32, name="o", tag="o")
                        nc.scalar.mul(o[:], s[:], gsc[:])
                        nc.sync.dma_start(out_d[g * P:(g + 1) * P, :], o[:])
                        state[k] = None  # release python refs

    nc.compile()
    return nc


def _get_program(bpc=BPC, reps=1):
    key = (bpc, reps)
    if key not in _PROGRAM_CACHE:
        _PROGRAM_CACHE[key] = _build_program(bpc, reps)
    return _PROGRAM_CACHE[key]


def _identity_np():
    return np.eye(P, dtype=np.float32)


def kernel(input_matrix: np.ndarray) -> np.ndarray:
    from concourse.bass_utils import run_bass_kernel_spmd

    x = np.ascontiguousarray(np.asarray(input_matrix, dtype=np.float32))
    assert x.shape == (B, R, H)
    nc = _get_program()
    ident = _identity_np()
    shards = x.reshape(N_CORES, BPC, R, H)
    in_maps = [
        {"x": np.ascontiguousarray(shards[i]), "ident": ident}
        for i in range(N_CORES)
    ]
    res = run_bass_kernel_spmd(nc, in_maps, core_ids=list(range(N_CORES)))
    out = np.concatenate(
        [res.results[i]["out"] for i in range(N_CORES)], axis=0
    )
    return out


if __name__ == "__main__":
    nc = _get_program()
    print("program built and compiled OK")
